# revision 8
# baseline (speedup 1.0000x reference)
"""Trainium2 Bass kernel for nn_ConvSparseKernel (sparse-tap conv, 5 taps).

Computation (per reference):
    Wn[k] = row-standardized W[k]  (per (k, out) row: subtract mean over in,
            then L2-normalize)
    y[b, :, oh, ow] = (sum_k Wn[k] @ x[b, :, oh+kh_k, ow+kw_k] + bias) * NF

Shapes (full): x [16, 256, 64, 64] f32, W [5, 256, 256] f32, bias [256] f32
Output: [16, 256, 62, 62] f32.

Sharding: data-parallel over batch -- 8 cores x 2 batches each; W/bias
replicated. Everything (standardization included) runs on-device.

Schedule (cost-model-driven; times are TimelineSim estimates):
  - PE must be continuously busy ~3us before it reaches 2.4 GHz, so a
    memset scratch tile feeds dummy transposes from ~0.35us (no identity
    dependency).  The real f32r matmul stream starts ~4.8us.
  - W taps load as 5 separate SWDGE DMAs on the gpsimd ring (25ns SEQ
    issue each); per-tap standardization (ACT square / DVE chain) runs as
    each tap lands, wn_k ready every ~1.0us.
  - x loads as row-pieces (20/14/14/16 rows) split ic0->SP ring,
    ic1->ACT ring, sized so each piece lands just before the matmul
    stream consumes it.  b1 x follows on the same rings.
  - prep0 runs tap-major blocks with growing chunk counts (t0/t1: c0-c1,
    t2-t4: c0-c2) matched to wn_k arrival; the PE transpose for tap k+1
    is tucked inside tap k's matmul block so its PSUM->SBUF copy never
    stalls the stream.
  - Main loop: per (b, oc, 8-row chunk) one PSUM bank accumulates 10
    f32r matmuls (N=496 -> 1 cycle/row); ACT applies bias*NF + scale and
    the y DMA drains on the gpsimd ring.  The last two drains use the
    ACT/SP HWDGE rings (625ns gen vs 1038ns SWDGE) to shorten the tail.
"""

import os

import numpy as np

KERNEL_KEYS = ((0, 0), (0, 2), (1, 1), (2, 0), (2, 2))
IN_CH = 256
OUT_CH = 256
H = 64
OH = 62
B_FULL = 16
N_CORES = 8
B_LOCAL = B_FULL // N_CORES
NF = float(1.0 / np.sqrt(IN_CH * len(KERNEL_KEYS) + 1))
ROW_CHUNK = 8  # rows of output per PSUM tile -> N = 8*62 = 496 <= 512

# x row-piece boundaries for batch 0 (streamed ahead of the matmuls) and
# batch 1 (bulk, latency-insensitive).  The DMA copy engine is a single
# serial ~360B/ns resource in the cost model, so pieces are sized to the
# matmul stream's consumption order.
ROWS_B0 = ((0, 10), (10, 18), (18, 34), (34, 50), (50, 64))
ROWS_B1 = ((0, 32), (32, 64))

# Dummy-transpose warmup calibration (see _emit).  PE SEQ issues these at
# ~233ns each, so the count directly sets when the real stream can start.
DUM_BIG = int(os.environ.get("DUM_BIG", "14"))
DUM_SPLICE = int(os.environ.get("DUM_SPLICE", "2"))

_compiled_nc = None


def _emit(tc, nc, y, x, w, bias):
    import concourse.mybir as mybir
    from concourse.masks import make_identity

    f32 = mybir.dt.float32
    f32r = mybir.dt.float32r
    AF = mybir.ActivationFunctionType
    AX = mybir.AxisListType
    NTAP = len(KERNEL_KEYS)

    w_okI = w.rearrange("k o i -> o k i")
    bias2d = bias.rearrange("(p u) -> p u", u=1)

    with tc.tile_pool(name="const", bufs=1) as cpool, \
         tc.tile_pool(name="wprep", bufs=1) as wpool, \
         tc.tile_pool(name="tpsum", bufs=2, space="PSUM") as tpool, \
         tc.tile_pool(name="mmpsum", bufs=5, space="PSUM") as mpool, \
         tc.tile_pool(name="outp", bufs=12) as opool:

        # ---- SBUF tiles ----
        junk = cpool.tile([64, 64], f32, name="junk")
        ident_f32 = cpool.tile([128, 128], f32, name="ident_f32")
        ident = cpool.tile([128, 128], f32r, name="ident")
        sqrt_warm = cpool.tile([64, 1], f32, name="sqrt_warm")
        wraw = [cpool.tile([128, NTAP, IN_CH], f32, name=f"wraw_{oc}",
                           tag=f"wraw_{oc}") for oc in range(2)]
        braw = [cpool.tile([128, 1], f32, name=f"braw_{oc}",
                           tag=f"braw_{oc}") for oc in range(2)]
        bnf = [cpool.tile([128, 1], f32, name=f"bnf_{oc}", tag=f"bnf_{oc}")
               for oc in range(2)]
        wn = [wpool.tile([128, NTAP, IN_CH], f32r, name=f"wn_{oc}",
                         tag=f"wn_{oc}") for oc in range(2)]
        wt = [cpool.tile([128, NTAP, 2, 128], f32r, name=f"wt_{oc}",
                         tag=f"wt_{oc}") for oc in range(2)]
        xt = [[cpool.tile([128, H, H], f32r, name=f"xt_{b}_{cc}",
                          tag=f"xt_{b}_{cc}") for cc in range(2)]
              for b in range(B_LOCAL)]
        st = {}
        for oc in range(2):
            for nm in ("ssq", "sums", "mu", "musums", "var", "sd", "inv"):
                st[(oc, nm)] = wpool.tile([128, NTAP], f32,
                                          name=f"{nm}_{oc}",
                                          tag=f"{nm}_{oc}")
            st[(oc, "sqs")] = wpool.tile([128, IN_CH], f32,
                                         name=f"sqs_{oc}", tag=f"sqs_{oc}")

        # ---- Pool (gpsimd/SWDGE) ring: scratch memset, W tap0 (its own
        # descriptor generator, 25ns SEQ -> earliest possible data),
        # identity build, remaining W taps/bias, later the y drains.
        nc.gpsimd.memset(junk, 1.0)
        nc.gpsimd.dma_start(out=wraw[0][:, 0, :], in_=w_okI[0:128, 0, :])
        make_identity(nc, ident_f32)
        for k in range(1, NTAP):
            nc.gpsimd.dma_start(out=wraw[0][:, k, :], in_=w_okI[0:128, k, :])
        nc.gpsimd.dma_start(out=braw[0], in_=bias2d[0:128])
        nc.gpsimd.dma_start(out=wraw[1], in_=w_okI[128:256])
        nc.gpsimd.dma_start(out=braw[1], in_=bias2d[128:256])

        # ---- ACT (HWDGE) ring: sqrt-table warm first, then b0.ic1 row
        # pieces; b1.ic1 comes later (emitted after the oc0 stats) so its
        # copies queue behind the b0 pieces on the shared copy engine.
        nc.scalar.sqrt(sqrt_warm, junk[:, 0:1])
        for (r0, r1) in ROWS_B0:
            nc.scalar.dma_start(out=xt[0][1][:, r0:r1, :],
                                in_=x[0, 128:256, r0:r1, :])

        # ---- SP (HWDGE) ring: b0.ic0 pieces.  b1.ic0 is emitted later,
        # behind a mid-stream drain, so its bulk copies cannot overtake
        # the b0 pieces in the global copy queue.  The very last y drain
        # is appended at the end of emission.
        for (r0, r1) in ROWS_B0:
            nc.sync.dma_start(out=xt[0][0][:, r0:r1, :],
                              in_=x[0, 0:128, r0:r1, :])

        # ---- DVE: identity f32->f32r round-copy first (ident_f32 lands
        # ~1.9us; real transposes need it ~4.1us), then the stats chains.
        nc.vector.tensor_copy(out=ident, in_=ident_f32)

        # ---- PE warmup: dummy transposes on the memset scratch keep PE
        # continuously busy from ~0.35us so the p-state ramp (3us) is done
        # before the real stream starts.  [64,64] f32 transposes are 128
        # PE cycles each; the [32,32] tail gives fine splice granularity.
        def dummy(n, small=False):
            for _ in range(n):
                dt_ = tpool.tile([64, 64], f32, name="dum", tag="dum",
                                 bufs=1)
                if small:
                    nc.tensor.transpose(dt_[0:32, 0:32], junk[0:32, 0:32],
                                        junk[0:32, 0:32])
                else:
                    nc.tensor.transpose(dt_, junk, junk)

        dummy(DUM_BIG)

        # ---- weight standardization (per tap) ----
        # ||w - mu||^2 = ssq - mu*sums, so sq/ssq don't wait on the mean.
        def stats_tap(oc, k):
            ks = slice(k, k + 1)
            # ssq_k = sum(w_k^2) on ACT (Square + accum), off the DVE
            # chain. (tensor_tensor_reduce wedges TRN2 here.)
            nc.scalar.activation(st[(oc, "sqs")], wraw[oc][:, k, :],
                                 AF.Square, accum_out=st[(oc, "ssq")][:, ks])
            nc.vector.reduce_sum(out=st[(oc, "sums")][:, ks],
                                 in_=wraw[oc][:, k, :], axis=AX.X)
            nc.vector.tensor_scalar_mul(st[(oc, "mu")][:, ks],
                                        st[(oc, "sums")][:, ks], 1.0 / IN_CH)
            nc.vector.tensor_mul(out=st[(oc, "musums")][:, ks],
                                 in0=st[(oc, "mu")][:, ks],
                                 in1=st[(oc, "sums")][:, ks])
            nc.vector.tensor_sub(out=st[(oc, "var")][:, ks],
                                 in0=st[(oc, "ssq")][:, ks],
                                 in1=st[(oc, "musums")][:, ks])
            nc.scalar.sqrt(st[(oc, "sd")][:, ks], st[(oc, "var")][:, ks])
            nc.vector.reciprocal(st[(oc, "inv")][:, ks],
                                 st[(oc, "sd")][:, ks])
            # wn_k = (w_k - mu_k) * inv_k, one fused DVE op
            nc.vector.tensor_scalar(
                out=wn[oc][:, k, :], in0=wraw[oc][:, k, :],
                scalar1=st[(oc, "mu")][:, ks],
                scalar2=st[(oc, "inv")][:, ks],
                op0=mybir.AluOpType.subtract,
                op1=mybir.AluOpType.mult)

        def transpose_tap(oc, k):
            for ic in range(2):
                pt = tpool.tile([128, 128], f32r, name="pt", tag="pt")
                nc.tensor.transpose(
                    pt, wn[oc][:, k, ic * 128:(ic + 1) * 128], ident)
                # alternate PSUM->SBUF copy engine: DVE / ACT
                if ic == 0:
                    nc.vector.tensor_copy(out=wt[oc][:, k, ic, :], in_=pt)
                else:
                    nc.scalar.copy(wt[oc][:, k, ic, :], pt)

        # ---- main-loop helpers ----
        # Per-chunk PSUM tiles accumulate 10 matmuls; prep0 spreads them
        # across tap-major blocks so start/stop flags are tracked per
        # chunk.
        chunk_ps = {}
        chunk_cnt = {}

        def mm(b, oc, c, k, ic):
            key = (b, oc, c)
            r0 = c * ROW_CHUNK
            nr = min(ROW_CHUNK, OH - r0)
            if key not in chunk_ps:
                chunk_ps[key] = mpool.tile([128, nr, OH], f32, name="ps",
                                           tag="ps")
                chunk_cnt[key] = 0
            idx = chunk_cnt[key]
            kh, kw = KERNEL_KEYS[k]
            rhs = xt[b][ic][:, kh + r0:kh + r0 + nr, kw:kw + OH]
            nc.tensor.matmul(chunk_ps[key], wt[oc][:, k, ic, :], rhs,
                             start=(idx == 0), stop=(idx == 2 * NTAP - 1))
            chunk_cnt[key] = idx + 1

        def drain_chunk(b, oc, c, ring=None):
            key = (b, oc, c)
            assert chunk_cnt[key] == 2 * NTAP
            r0 = c * ROW_CHUNK
            nr = min(ROW_CHUNK, OH - r0)
            ot = opool.tile([128, nr, OH], f32, name="ot", tag="ot")
            nc.scalar.activation(ot, chunk_ps[key], AF.Identity,
                                 bias=bnf[oc], scale=NF)
            eng = ring or nc.gpsimd
            eng.dma_start(
                out=y[b, oc * 128:(oc + 1) * 128, r0:r0 + nr, :], in_=ot)
            del chunk_ps[key], chunk_cnt[key]

        def conv_chunk(b, oc, c, ring=None):
            for k in range(NTAP):
                for ic in range(2):
                    mm(b, oc, c, k, ic)
            drain_chunk(b, oc, c, ring=ring)

        NCH = (OH + ROW_CHUNK - 1) // ROW_CHUNK  # 8 chunks (last is 6 rows)

        # ---- prep0: oc0 stats/transposes fused with the first chunks'
        # matmuls.  Tap blocks cover c0/c1 in x-arrival order (ic0 piece
        # lands before ic1); the transpose for tap k+1 is tucked inside
        # tap k's block so its PSUM->SBUF copy hides under matmuls.  c2
        # runs as one block at the end (ic0 first: its piece lands ~1.5us
        # before the ic1 one).  Emission order = per-engine program order.
        stats_tap(0, 0)
        transpose_tap(0, 0)
        dummy(DUM_SPLICE)            # covers the wt0 PSUM->SBUF copy
        stats_tap(0, 1)
        stats_tap(0, 2)
        for k in range(NTAP):
            mm(0, 0, 0, k, 0)
            mm(0, 0, 0, k, 1)
            if k + 1 < NTAP:
                transpose_tap(0, k + 1)
            if k == 0:
                stats_tap(0, 3)
            if k == 1:
                stats_tap(0, 4)
            mm(0, 0, 1, k, 0)
            mm(0, 0, 1, k, 1)
        # c2 as one block, ic0 sweep then ic1 sweep
        for ic in range(2):
            for k in range(NTAP):
                mm(0, 0, 2, k, ic)
        # bnf0 on ACT before the first drain activation
        nc.scalar.mul(bnf[0], braw[0], NF)
        drain_chunk(0, 0, 0)
        drain_chunk(0, 0, 1)
        # c2's drain goes out on the SP ring: it also acts as a timed
        # barrier so the b1.ic0 bulk DMAs (emitted right after it) cannot
        # enter the global copy queue before the b0 pieces have drained.
        drain_chunk(0, 0, 2, ring=nc.sync)
        for (r0, r1) in ROWS_B1:
            nc.sync.dma_start(out=xt[1][0][:, r0:r1, :],
                              in_=x[1, 0:128, r0:r1, :])

        # b1.ic1 bulk loads ride the ACT ring behind the b0.ic1 pieces.
        for (r0, r1) in ROWS_B1:
            nc.scalar.dma_start(out=xt[1][1][:, r0:r1, :],
                                in_=x[1, 128:256, r0:r1, :])
        nc.scalar.mul(bnf[1], braw[1], NF)

        # oc1 stats (W oc1 lands ~18us; transposes are interleaved into
        # the c3..c7 chunk stream below).
        for k in range(NTAP):
            stats_tap(1, k)

        # rest of b0.oc0, with oc1 transposes tucked between chunks
        for c in range(3, NCH):
            conv_chunk(0, 0, c)
            if c - 3 < NTAP:
                transpose_tap(1, c - 3)

        for c in range(NCH):
            conv_chunk(0, 1, c)
        for c in range(NCH):
            conv_chunk(1, 0, c)
        for c in range(NCH):
            ring = None
            if c == NCH - 2:
                ring = nc.scalar
            elif c == NCH - 1:
                ring = nc.sync
            conv_chunk(1, 1, c, ring=ring)


def _build_nc():
    import concourse.mybir as mybir
    import concourse.tile as tile
    from concourse import bacc

    f32 = mybir.dt.float32
    f32r = mybir.dt.float32r
    nc = bacc.Bacc("TRN2", target_bir_lowering=False, debug=False)
    x = nc.dram_tensor("x", (B_LOCAL, IN_CH, H, H), f32r,
                       kind="ExternalInput").ap()
    w = nc.dram_tensor("w", (len(KERNEL_KEYS), OUT_CH, IN_CH), f32,
                       kind="ExternalInput").ap()
    bias = nc.dram_tensor("bias", (OUT_CH,), f32, kind="ExternalInput").ap()
    y = nc.dram_tensor("y", (B_LOCAL, OUT_CH, OH, OH), f32,
                       kind="ExternalOutput").ap()

    with tile.TileContext(nc) as tc:
        _emit(tc, nc, y, x, w, bias)
    nc.compile()
    return nc


def _get_nc():
    global _compiled_nc
    if _compiled_nc is None:
        _compiled_nc = _build_nc()
    return _compiled_nc


def _make_in_maps(x, W, bias):
    x = np.ascontiguousarray(x, dtype=np.float32)
    W = np.ascontiguousarray(W, dtype=np.float32)
    bias = np.ascontiguousarray(bias, dtype=np.float32)
    return [
        {
            "x": np.ascontiguousarray(x[i * B_LOCAL:(i + 1) * B_LOCAL]),
            "w": W,
            "bias": bias,
        }
        for i in range(N_CORES)
    ]


def kernel(x, W, bias):
    from concourse import bass_utils

    nc = _get_nc()
    res = bass_utils.run_bass_kernel_spmd(
        nc, _make_in_maps(x, W, bias), core_ids=list(range(N_CORES)))
    return np.concatenate([r["y"] for r in res.results], axis=0)


# revision 11
# speedup vs baseline: 1.0921x; 1.0921x over previous
"""Trainium2 Bass kernel for nn_ConvSparseKernel (sparse-tap conv, 5 taps).

Computation (per reference):
    Wn[k] = row-standardized W[k]  (per (k, out) row: subtract mean over in,
            then L2-normalize)
    y[b, :, oh, ow] = (sum_k Wn[k] @ x[b, :, oh+kh_k, ow+kw_k] + bias) * NF

Shapes (full): x [16, 256, 64, 64] f32, W [5, 256, 256] f32, bias [256] f32
Output: [16, 256, 62, 62] f32.

Sharding: data-parallel over batch -- 8 cores x 2 batches each; W/bias
replicated. Everything (standardization included) runs on-device.

Schedule (cost-model-driven; times are TimelineSim estimates):
  - PE must be continuously busy ~3us before it reaches 2.4 GHz, so a
    memset scratch tile feeds dummy transposes from ~0.35us (no identity
    dependency).  The real f32r matmul stream starts ~4.8us.
  - W taps load as 5 separate SWDGE DMAs on the gpsimd ring (25ns SEQ
    issue each); per-tap standardization (ACT square / DVE chain) runs as
    each tap lands, wn_k ready every ~1.0us.
  - x loads as row-pieces (20/14/14/16 rows) split ic0->SP ring,
    ic1->ACT ring, sized so each piece lands just before the matmul
    stream consumes it.  b1 x follows on the same rings.
  - prep0 runs tap-major blocks with growing chunk counts (t0/t1: c0-c1,
    t2-t4: c0-c2) matched to wn_k arrival; the PE transpose for tap k+1
    is tucked inside tap k's matmul block so its PSUM->SBUF copy never
    stalls the stream.
  - Main loop: per (b, oc, 8-row chunk) one PSUM bank accumulates 10
    f32r matmuls (N=496 -> 1 cycle/row); ACT applies bias*NF + scale and
    the y DMA drains on the gpsimd ring.  The last two drains use the
    ACT/SP HWDGE rings (625ns gen vs 1038ns SWDGE) to shorten the tail.
"""

import os

import numpy as np

KERNEL_KEYS = ((0, 0), (0, 2), (1, 1), (2, 0), (2, 2))
IN_CH = 256
OUT_CH = 256
H = 64
OH = 62
B_FULL = 16
N_CORES = 8
B_LOCAL = B_FULL // N_CORES
NF = float(1.0 / np.sqrt(IN_CH * len(KERNEL_KEYS) + 1))
ROW_CHUNK = 8  # rows of output per PSUM tile -> N = 8*62 = 496 <= 512

# x row-piece boundaries for batch 0 (streamed ahead of the matmuls) and
# batch 1 (bulk, latency-insensitive).  The DMA copy engine is a single
# serial ~360B/ns resource in the cost model, so pieces are sized to the
# matmul stream's consumption order.
ROWS_B0 = ((0, 10), (10, 18), (18, 34), (34, 50), (50, 64))
ROWS_B1 = ((0, 32), (32, 64))

# Dummy-transpose warmup calibration (see _emit).  PE SEQ issues these at
# ~233ns each, so the count directly sets when the real stream can start.
DUM_BIG = int(os.environ.get("DUM_BIG", "14"))
DUM_SPLICE = int(os.environ.get("DUM_SPLICE", "2"))

_compiled_nc = None


def _emit(tc, nc, y, x, w, bias):
    import concourse.mybir as mybir
    from concourse.masks import make_identity

    f32 = mybir.dt.float32
    f32r = mybir.dt.float32r
    AF = mybir.ActivationFunctionType
    AX = mybir.AxisListType
    NTAP = len(KERNEL_KEYS)

    w_okI = w.rearrange("k o i -> o k i")
    bias2d = bias.rearrange("(p u) -> p u", u=1)

    with tc.tile_pool(name="const", bufs=1) as cpool, \
         tc.tile_pool(name="wprep", bufs=1) as wpool, \
         tc.tile_pool(name="tpsum", bufs=2, space="PSUM") as tpool, \
         tc.tile_pool(name="mmpsum", bufs=5, space="PSUM") as mpool, \
         tc.tile_pool(name="outp", bufs=12) as opool:

        # ---- SBUF tiles ----
        junk = cpool.tile([64, 64], f32, name="junk")
        ident_f32 = cpool.tile([128, 128], f32, name="ident_f32")
        ident = cpool.tile([128, 128], f32r, name="ident")
        sqrt_warm = cpool.tile([64, 1], f32, name="sqrt_warm")
        wraw = [cpool.tile([128, NTAP, IN_CH], f32, name=f"wraw_{oc}",
                           tag=f"wraw_{oc}") for oc in range(2)]
        braw = [cpool.tile([128, 1], f32, name=f"braw_{oc}",
                           tag=f"braw_{oc}") for oc in range(2)]
        bnf = [cpool.tile([128, 1], f32, name=f"bnf_{oc}", tag=f"bnf_{oc}")
               for oc in range(2)]
        wn = [wpool.tile([128, NTAP, IN_CH], f32r, name=f"wn_{oc}",
                         tag=f"wn_{oc}") for oc in range(2)]
        wt = [cpool.tile([128, NTAP, 2, 128], f32r, name=f"wt_{oc}",
                         tag=f"wt_{oc}") for oc in range(2)]
        xt = [[cpool.tile([128, H, H], f32r, name=f"xt_{b}_{cc}",
                          tag=f"xt_{b}_{cc}") for cc in range(2)]
              for b in range(B_LOCAL)]
        st = {}
        for oc in range(2):
            for nm in ("ssq", "sums", "mu", "musums", "var", "sd", "inv"):
                st[(oc, nm)] = wpool.tile([128, NTAP], f32,
                                          name=f"{nm}_{oc}",
                                          tag=f"{nm}_{oc}")
            st[(oc, "sqs")] = wpool.tile([128, IN_CH], f32,
                                         name=f"sqs_{oc}", tag=f"sqs_{oc}")

        # ---- Pool (gpsimd/SWDGE) ring: scratch memset, W tap0 (its own
        # descriptor generator, 25ns SEQ -> earliest possible data),
        # identity build, remaining W taps/bias, later the y drains.
        nc.gpsimd.memset(junk, 1.0)
        nc.gpsimd.dma_start(out=wraw[0][:, 0, :], in_=w_okI[0:128, 0, :])
        make_identity(nc, ident_f32)
        for k in range(1, NTAP):
            nc.gpsimd.dma_start(out=wraw[0][:, k, :], in_=w_okI[0:128, k, :])
        nc.gpsimd.dma_start(out=braw[0], in_=bias2d[0:128])
        nc.gpsimd.dma_start(out=wraw[1], in_=w_okI[128:256])
        nc.gpsimd.dma_start(out=braw[1], in_=bias2d[128:256])

        # ---- ACT (HWDGE) ring: sqrt-table warm first, then the first two
        # b0.ic1 row pieces.  The rest (q2..q4, b1.ic1) are emitted later:
        # each dma issue hogs ACT.SEQ for ~1.2us (HWDGE backpressure) and
        # would delay the stats Square/sqrt launches.
        nc.scalar.sqrt(sqrt_warm, junk[:, 0:1])
        for (r0, r1) in ROWS_B0[:2]:
            nc.scalar.dma_start(out=xt[0][1][:, r0:r1, :],
                                in_=x[0, 128:256, r0:r1, :])

        # ---- SP (HWDGE) ring: b0.ic0 pieces.  b1.ic0 is emitted later,
        # behind a mid-stream drain, so its bulk copies cannot overtake
        # the b0 pieces in the global copy queue.  The very last y drain
        # is appended at the end of emission.
        for (r0, r1) in ROWS_B0:
            nc.sync.dma_start(out=xt[0][0][:, r0:r1, :],
                              in_=x[0, 0:128, r0:r1, :])

        # ---- DVE: identity f32->f32r round-copy first (ident_f32 lands
        # ~1.9us; real transposes need it ~4.1us), then the stats chains.
        nc.vector.tensor_copy(out=ident, in_=ident_f32)

        # ---- PE warmup: dummy transposes on the memset scratch keep PE
        # continuously busy from ~0.35us so the p-state ramp (3us) is done
        # before the real stream starts.  [64,64] f32 transposes are 128
        # PE cycles each; the [32,32] tail gives fine splice granularity.
        def dummy(n, small=False):
            for _ in range(n):
                dt_ = tpool.tile([64, 64], f32, name="dum", tag="dum",
                                 bufs=1)
                if small:
                    nc.tensor.transpose(dt_[0:32, 0:32], junk[0:32, 0:32],
                                        junk[0:32, 0:32])
                else:
                    nc.tensor.transpose(dt_, junk, junk)

        dummy(DUM_BIG)

        # ---- weight standardization (per tap) ----
        # ||w - mu||^2 = ssq - mu*sums, so sq/ssq don't wait on the mean.
        def stats_tap(oc, k):
            ks = slice(k, k + 1)
            # ssq_k = sum(w_k^2) on ACT (Square + accum), off the DVE
            # chain. (tensor_tensor_reduce wedges TRN2 here.)
            nc.scalar.activation(st[(oc, "sqs")], wraw[oc][:, k, :],
                                 AF.Square, accum_out=st[(oc, "ssq")][:, ks])
            nc.vector.reduce_sum(out=st[(oc, "sums")][:, ks],
                                 in_=wraw[oc][:, k, :], axis=AX.X)
            nc.vector.tensor_scalar_mul(st[(oc, "mu")][:, ks],
                                        st[(oc, "sums")][:, ks], 1.0 / IN_CH)
            nc.vector.tensor_mul(out=st[(oc, "musums")][:, ks],
                                 in0=st[(oc, "mu")][:, ks],
                                 in1=st[(oc, "sums")][:, ks])
            nc.vector.tensor_sub(out=st[(oc, "var")][:, ks],
                                 in0=st[(oc, "ssq")][:, ks],
                                 in1=st[(oc, "musums")][:, ks])
            nc.scalar.sqrt(st[(oc, "sd")][:, ks], st[(oc, "var")][:, ks])
            nc.vector.reciprocal(st[(oc, "inv")][:, ks],
                                 st[(oc, "sd")][:, ks])
            # wn_k = (w_k - mu_k) * inv_k, one fused DVE op
            nc.vector.tensor_scalar(
                out=wn[oc][:, k, :], in0=wraw[oc][:, k, :],
                scalar1=st[(oc, "mu")][:, ks],
                scalar2=st[(oc, "inv")][:, ks],
                op0=mybir.AluOpType.subtract,
                op1=mybir.AluOpType.mult)

        def transpose_tap(oc, k):
            for ic in range(2):
                pt = tpool.tile([128, 128], f32r, name="pt", tag="pt")
                nc.tensor.transpose(
                    pt, wn[oc][:, k, ic * 128:(ic + 1) * 128], ident)
                # alternate PSUM->SBUF copy engine: DVE / ACT
                if ic == 0:
                    nc.vector.tensor_copy(out=wt[oc][:, k, ic, :], in_=pt)
                else:
                    nc.scalar.copy(wt[oc][:, k, ic, :], pt)

        # ---- main-loop helpers ----
        # Per-chunk PSUM tiles accumulate 10 matmuls; prep0 spreads them
        # across tap-major blocks so start/stop flags are tracked per
        # chunk.
        chunk_ps = {}
        chunk_cnt = {}

        def mm(b, oc, c, k, ic):
            key = (b, oc, c)
            r0 = c * ROW_CHUNK
            nr = min(ROW_CHUNK, OH - r0)
            if key not in chunk_ps:
                chunk_ps[key] = mpool.tile([128, nr, OH], f32, name="ps",
                                           tag="ps")
                chunk_cnt[key] = 0
            idx = chunk_cnt[key]
            kh, kw = KERNEL_KEYS[k]
            rhs = xt[b][ic][:, kh + r0:kh + r0 + nr, kw:kw + OH]
            nc.tensor.matmul(chunk_ps[key], wt[oc][:, k, ic, :], rhs,
                             start=(idx == 0), stop=(idx == 2 * NTAP - 1))
            chunk_cnt[key] = idx + 1

        def drain_chunk(b, oc, c, ring=None):
            key = (b, oc, c)
            assert chunk_cnt[key] == 2 * NTAP
            r0 = c * ROW_CHUNK
            nr = min(ROW_CHUNK, OH - r0)
            ot = opool.tile([128, nr, OH], f32, name="ot", tag="ot")
            nc.scalar.activation(ot, chunk_ps[key], AF.Identity,
                                 bias=bnf[oc], scale=NF)
            eng = ring or nc.gpsimd
            eng.dma_start(
                out=y[b, oc * 128:(oc + 1) * 128, r0:r0 + nr, :], in_=ot)
            del chunk_ps[key], chunk_cnt[key]

        def conv_chunk(b, oc, c, ring=None):
            for k in range(NTAP):
                for ic in range(2):
                    mm(b, oc, c, k, ic)
            drain_chunk(b, oc, c, ring=ring)

        NCH = (OH + ROW_CHUNK - 1) // ROW_CHUNK  # 8 chunks (last is 6 rows)

        # ---- prep0: oc0 stats/transposes fused with the first chunks'
        # matmuls.  Tap blocks cover c0/c1 in x-arrival order (ic0 piece
        # lands before ic1); the transpose for tap k+1 is tucked inside
        # tap k's block so its PSUM->SBUF copy hides under matmuls.  c2
        # runs as one block at the end (ic0 first: its piece lands ~1.5us
        # before the ic1 one).  Emission order = per-engine program order.
        stats_tap(0, 0)
        stats_tap(0, 1)
        nc.scalar.dma_start(out=xt[0][1][:, ROWS_B0[2][0]:ROWS_B0[2][1], :],
                            in_=x[0, 128:256, ROWS_B0[2][0]:ROWS_B0[2][1], :])
        stats_tap(0, 2)
        stats_tap(0, 3)
        nc.scalar.dma_start(out=xt[0][1][:, ROWS_B0[3][0]:ROWS_B0[3][1], :],
                            in_=x[0, 128:256, ROWS_B0[3][0]:ROWS_B0[3][1], :])
        stats_tap(0, 4)
        nc.scalar.dma_start(out=xt[0][1][:, ROWS_B0[4][0]:ROWS_B0[4][1], :],
                            in_=x[0, 128:256, ROWS_B0[4][0]:ROWS_B0[4][1], :])
        transpose_tap(0, 0)
        dummy(DUM_SPLICE)            # covers the wt0 PSUM->SBUF copy
        for k in range(NTAP):
            mm(0, 0, 0, k, 0)
            mm(0, 0, 0, k, 1)
            if k + 1 < NTAP:
                transpose_tap(0, k + 1)
            mm(0, 0, 1, k, 0)
            mm(0, 0, 1, k, 1)
        # c2 as one block, ic0 sweep then ic1 sweep
        for ic in range(2):
            for k in range(NTAP):
                mm(0, 0, 2, k, ic)
        # bnf0 on ACT before the first drain activation
        nc.scalar.mul(bnf[0], braw[0], NF)
        drain_chunk(0, 0, 0)
        drain_chunk(0, 0, 1)
        # c2's drain goes out on the SP ring: it also acts as a timed
        # barrier so the b1.ic0 bulk DMAs (emitted right after it) cannot
        # enter the global copy queue before the b0 pieces have drained.
        drain_chunk(0, 0, 2, ring=nc.sync)
        for (r0, r1) in ROWS_B1:
            nc.sync.dma_start(out=xt[1][0][:, r0:r1, :],
                              in_=x[1, 0:128, r0:r1, :])

        # b1.ic1 bulk loads ride the ACT ring behind the b0.ic1 pieces.
        for (r0, r1) in ROWS_B1:
            nc.scalar.dma_start(out=xt[1][1][:, r0:r1, :],
                                in_=x[1, 128:256, r0:r1, :])
        nc.scalar.mul(bnf[1], braw[1], NF)

        # rest of b0.oc0, with oc1 stats/transposes tucked between chunks.
        # The oc1 stats are emitted here (not earlier): the Tile scheduler
        # reorders within a window, and oc1 ops emitted near the oc0 chain
        # get hoisted ahead of it on the in-order DVE/ACT engines, gating
        # everything on W.oc1's late arrival.
        for c in range(3, NCH):
            if c - 3 < NTAP:
                stats_tap(1, c - 3)
            conv_chunk(0, 0, c)
            if c - 3 < NTAP:
                transpose_tap(1, c - 3)

        for c in range(NCH):
            conv_chunk(0, 1, c)
        for c in range(NCH):
            conv_chunk(1, 0, c)
        for c in range(NCH):
            ring = None
            if c == NCH - 2:
                ring = nc.scalar
            elif c == NCH - 1:
                ring = nc.sync
            conv_chunk(1, 1, c, ring=ring)


def _build_nc():
    import concourse.mybir as mybir
    import concourse.tile as tile
    from concourse import bacc

    f32 = mybir.dt.float32
    f32r = mybir.dt.float32r
    nc = bacc.Bacc("TRN2", target_bir_lowering=False, debug=False)
    x = nc.dram_tensor("x", (B_LOCAL, IN_CH, H, H), f32r,
                       kind="ExternalInput").ap()
    w = nc.dram_tensor("w", (len(KERNEL_KEYS), OUT_CH, IN_CH), f32,
                       kind="ExternalInput").ap()
    bias = nc.dram_tensor("bias", (OUT_CH,), f32, kind="ExternalInput").ap()
    y = nc.dram_tensor("y", (B_LOCAL, OUT_CH, OH, OH), f32,
                       kind="ExternalOutput").ap()

    with tile.TileContext(nc) as tc:
        _emit(tc, nc, y, x, w, bias)
    nc.compile()
    return nc


def _get_nc():
    global _compiled_nc
    if _compiled_nc is None:
        _compiled_nc = _build_nc()
    return _compiled_nc


def _make_in_maps(x, W, bias):
    x = np.ascontiguousarray(x, dtype=np.float32)
    W = np.ascontiguousarray(W, dtype=np.float32)
    bias = np.ascontiguousarray(bias, dtype=np.float32)
    return [
        {
            "x": np.ascontiguousarray(x[i * B_LOCAL:(i + 1) * B_LOCAL]),
            "w": W,
            "bias": bias,
        }
        for i in range(N_CORES)
    ]


def kernel(x, W, bias):
    from concourse import bass_utils

    nc = _get_nc()
    res = bass_utils.run_bass_kernel_spmd(
        nc, _make_in_maps(x, W, bias), core_ids=list(range(N_CORES)))
    return np.concatenate([r["y"] for r in res.results], axis=0)


# revision 13
# speedup vs baseline: 1.1236x; 1.0289x over previous
"""Trainium2 Bass kernel for nn_ConvSparseKernel (sparse-tap conv, 5 taps).

Computation (per reference):
    Wn[k] = row-standardized W[k]  (per (k, out) row: subtract mean over in,
            then L2-normalize)
    y[b, :, oh, ow] = (sum_k Wn[k] @ x[b, :, oh+kh_k, ow+kw_k] + bias) * NF

Shapes (full): x [16, 256, 64, 64] f32, W [5, 256, 256] f32, bias [256] f32
Output: [16, 256, 62, 62] f32.

Sharding: data-parallel over batch -- 8 cores x 2 batches each; W/bias
replicated. Everything (standardization included) runs on-device.

Schedule (cost-model-driven; times are TimelineSim estimates):
  - PE must be continuously busy ~3us before it reaches 2.4 GHz, so a
    memset scratch tile feeds dummy transposes from ~0.35us (no identity
    dependency).  The real f32r matmul stream starts ~4.8us.
  - W taps load as 5 separate SWDGE DMAs on the gpsimd ring (25ns SEQ
    issue each); per-tap standardization (ACT square / DVE chain) runs as
    each tap lands, wn_k ready every ~1.0us.
  - x loads as row-pieces (20/14/14/16 rows) split ic0->SP ring,
    ic1->ACT ring, sized so each piece lands just before the matmul
    stream consumes it.  b1 x follows on the same rings.
  - prep0 runs tap-major blocks with growing chunk counts (t0/t1: c0-c1,
    t2-t4: c0-c2) matched to wn_k arrival; the PE transpose for tap k+1
    is tucked inside tap k's matmul block so its PSUM->SBUF copy never
    stalls the stream.
  - Main loop: per (b, oc, 8-row chunk) one PSUM bank accumulates 10
    f32r matmuls (N=496 -> 1 cycle/row); ACT applies bias*NF + scale and
    the y DMA drains on the gpsimd ring.  The last two drains use the
    ACT/SP HWDGE rings (625ns gen vs 1038ns SWDGE) to shorten the tail.
"""

import os

import numpy as np

KERNEL_KEYS = ((0, 0), (0, 2), (1, 1), (2, 0), (2, 2))
IN_CH = 256
OUT_CH = 256
H = 64
OH = 62
B_FULL = 16
N_CORES = 8
B_LOCAL = B_FULL // N_CORES
NF = float(1.0 / np.sqrt(IN_CH * len(KERNEL_KEYS) + 1))
ROW_CHUNK = 8  # rows of output per PSUM tile -> N = 8*62 = 496 <= 512

# x row-piece boundaries for batch 0 (streamed ahead of the matmuls) and
# batch 1 (bulk, latency-insensitive).  The DMA copy engine is a single
# serial ~360B/ns resource in the cost model, so pieces are sized to the
# matmul stream's consumption order.
ROWS_B0 = ((0, 10), (10, 18), (18, 34), (34, 50), (50, 64))
ROWS_B1 = ((0, 32), (32, 64))

# Dummy-transpose warmup calibration (see _emit).  PE SEQ issues these at
# ~233ns each, so the count directly sets when the real stream can start.
DUM_BIG = int(os.environ.get("DUM_BIG", "14"))
DUM_SPLICE = int(os.environ.get("DUM_SPLICE", "2"))

_compiled_nc = None


def _emit(tc, nc, y, x, w, bias):
    import concourse.mybir as mybir
    from concourse.masks import make_identity

    f32 = mybir.dt.float32
    f32r = mybir.dt.float32r
    AF = mybir.ActivationFunctionType
    AX = mybir.AxisListType
    NTAP = len(KERNEL_KEYS)

    w_okI = w.rearrange("k o i -> o k i")
    bias2d = bias.rearrange("(p u) -> p u", u=1)

    with tc.tile_pool(name="const", bufs=1) as cpool, \
         tc.tile_pool(name="wprep", bufs=1) as wpool, \
         tc.tile_pool(name="tpsum", bufs=2, space="PSUM") as tpool, \
         tc.tile_pool(name="mmpsum", bufs=5, space="PSUM") as mpool, \
         tc.tile_pool(name="outp", bufs=12) as opool:

        # ---- SBUF tiles ----
        junk = cpool.tile([64, 64], f32, name="junk")
        ident_f32 = cpool.tile([128, 128], f32, name="ident_f32")
        ident = cpool.tile([128, 128], f32r, name="ident")
        sqrt_warm = cpool.tile([64, 1], f32, name="sqrt_warm")
        wraw = [cpool.tile([128, NTAP, IN_CH], f32, name=f"wraw_{oc}",
                           tag=f"wraw_{oc}") for oc in range(2)]
        braw = [cpool.tile([128, 1], f32, name=f"braw_{oc}",
                           tag=f"braw_{oc}") for oc in range(2)]
        bnf = [cpool.tile([128, 1], f32, name=f"bnf_{oc}", tag=f"bnf_{oc}")
               for oc in range(2)]
        wn = [wpool.tile([128, NTAP, IN_CH], f32r, name=f"wn_{oc}",
                         tag=f"wn_{oc}") for oc in range(2)]
        wt = [cpool.tile([128, NTAP, 2, 128], f32r, name=f"wt_{oc}",
                         tag=f"wt_{oc}") for oc in range(2)]
        xt = [[cpool.tile([128, H, H], f32r, name=f"xt_{b}_{cc}",
                          tag=f"xt_{b}_{cc}") for cc in range(2)]
              for b in range(B_LOCAL)]
        st = {}
        for oc in range(2):
            for nm in ("ssq", "sums", "mu", "musums", "var", "sd", "inv"):
                st[(oc, nm)] = wpool.tile([128, NTAP], f32,
                                          name=f"{nm}_{oc}",
                                          tag=f"{nm}_{oc}")
            st[(oc, "sqs")] = wpool.tile([128, IN_CH], f32,
                                         name=f"sqs_{oc}", tag=f"sqs_{oc}")

        # ---- Pool (gpsimd/SWDGE) ring: scratch memset, W tap0 (its own
        # descriptor generator, 25ns SEQ -> earliest possible data),
        # identity build, remaining W taps/bias, later the y drains.
        nc.gpsimd.memset(junk, 1.0)
        nc.gpsimd.dma_start(out=wraw[0][:, 0, :], in_=w_okI[0:128, 0, :])
        make_identity(nc, ident_f32)
        # taps 1-4 in one DMA: a single copy-queue contender lands all
        # remaining oc0 taps by ~6.5us (split taps lost the FIFO race)
        nc.gpsimd.dma_start(out=wraw[0][:, 1:NTAP, :],
                            in_=w_okI[0:128, 1:NTAP, :])
        nc.gpsimd.dma_start(out=braw[0], in_=bias2d[0:128])
        nc.gpsimd.dma_start(out=wraw[1], in_=w_okI[128:256])
        nc.gpsimd.dma_start(out=braw[1], in_=bias2d[128:256])

        # ---- ACT (HWDGE) ring: sqrt-table warm first, then the first two
        # b0.ic1 row pieces.  The rest (q2..q4, b1.ic1) are emitted later:
        # each dma issue hogs ACT.SEQ for ~1.2us (HWDGE backpressure) and
        # would delay the stats Square/sqrt launches.
        nc.scalar.sqrt(sqrt_warm, junk[:, 0:1])
        for (r0, r1) in ROWS_B0[:2]:
            nc.scalar.dma_start(out=xt[0][1][:, r0:r1, :],
                                in_=x[0, 128:256, r0:r1, :])

        # ---- SP (HWDGE) ring: b0.ic0 pieces.  b1.ic0 is emitted later,
        # behind a mid-stream drain, so its bulk copies cannot overtake
        # the b0 pieces in the global copy queue.  The very last y drain
        # is appended at the end of emission.
        for (r0, r1) in ROWS_B0:
            nc.sync.dma_start(out=xt[0][0][:, r0:r1, :],
                              in_=x[0, 0:128, r0:r1, :])

        # ---- DVE: identity f32->f32r round-copy first (ident_f32 lands
        # ~1.9us; real transposes need it ~4.1us), then the stats chains.
        nc.vector.tensor_copy(out=ident, in_=ident_f32)

        # ---- PE warmup: dummy transposes on the memset scratch keep PE
        # continuously busy from ~0.35us so the p-state ramp (3us) is done
        # before the real stream starts.  [64,64] f32 transposes are 128
        # PE cycles each; the [32,32] tail gives fine splice granularity.
        def dummy(n, small=False):
            for _ in range(n):
                dt_ = tpool.tile([64, 64], f32, name="dum", tag="dum",
                                 bufs=1)
                if small:
                    nc.tensor.transpose(dt_[0:32, 0:32], junk[0:32, 0:32],
                                        junk[0:32, 0:32])
                else:
                    nc.tensor.transpose(dt_, junk, junk)

        dummy(DUM_BIG)

        # ---- weight standardization (per tap) ----
        # ||w - mu||^2 = ssq - mu*sums, so sq/ssq don't wait on the mean.
        def stats_tap(oc, k):
            ks = slice(k, k + 1)
            # ssq_k = sum(w_k^2) on ACT (Square + accum), off the DVE
            # chain. (tensor_tensor_reduce wedges TRN2 here.)
            nc.scalar.activation(st[(oc, "sqs")], wraw[oc][:, k, :],
                                 AF.Square, accum_out=st[(oc, "ssq")][:, ks])
            nc.vector.reduce_sum(out=st[(oc, "sums")][:, ks],
                                 in_=wraw[oc][:, k, :], axis=AX.X)
            nc.vector.tensor_scalar_mul(st[(oc, "mu")][:, ks],
                                        st[(oc, "sums")][:, ks], 1.0 / IN_CH)
            nc.vector.tensor_mul(out=st[(oc, "musums")][:, ks],
                                 in0=st[(oc, "mu")][:, ks],
                                 in1=st[(oc, "sums")][:, ks])
            nc.vector.tensor_sub(out=st[(oc, "var")][:, ks],
                                 in0=st[(oc, "ssq")][:, ks],
                                 in1=st[(oc, "musums")][:, ks])
            nc.scalar.sqrt(st[(oc, "sd")][:, ks], st[(oc, "var")][:, ks])
            nc.vector.reciprocal(st[(oc, "inv")][:, ks],
                                 st[(oc, "sd")][:, ks])
            # wn_k = (w_k - mu_k) * inv_k, one fused DVE op
            nc.vector.tensor_scalar(
                out=wn[oc][:, k, :], in0=wraw[oc][:, k, :],
                scalar1=st[(oc, "mu")][:, ks],
                scalar2=st[(oc, "inv")][:, ks],
                op0=mybir.AluOpType.subtract,
                op1=mybir.AluOpType.mult)

        def transpose_tap(oc, k):
            for ic in range(2):
                pt = tpool.tile([128, 128], f32r, name="pt", tag="pt")
                nc.tensor.transpose(
                    pt, wn[oc][:, k, ic * 128:(ic + 1) * 128], ident)
                # alternate PSUM->SBUF copy engine: DVE / ACT
                if ic == 0:
                    nc.vector.tensor_copy(out=wt[oc][:, k, ic, :], in_=pt)
                else:
                    nc.scalar.copy(wt[oc][:, k, ic, :], pt)

        # ---- main-loop helpers ----
        # Per-chunk PSUM tiles accumulate 10 matmuls; prep0 spreads them
        # across tap-major blocks so start/stop flags are tracked per
        # chunk.
        chunk_ps = {}
        chunk_cnt = {}

        def mm(b, oc, c, k, ic):
            key = (b, oc, c)
            r0 = c * ROW_CHUNK
            nr = min(ROW_CHUNK, OH - r0)
            if key not in chunk_ps:
                chunk_ps[key] = mpool.tile([128, nr, OH], f32, name="ps",
                                           tag="ps")
                chunk_cnt[key] = 0
            idx = chunk_cnt[key]
            kh, kw = KERNEL_KEYS[k]
            rhs = xt[b][ic][:, kh + r0:kh + r0 + nr, kw:kw + OH]
            nc.tensor.matmul(chunk_ps[key], wt[oc][:, k, ic, :], rhs,
                             start=(idx == 0), stop=(idx == 2 * NTAP - 1))
            chunk_cnt[key] = idx + 1

        def drain_chunk(b, oc, c, ring=None):
            key = (b, oc, c)
            assert chunk_cnt[key] == 2 * NTAP
            r0 = c * ROW_CHUNK
            nr = min(ROW_CHUNK, OH - r0)
            ot = opool.tile([128, nr, OH], f32, name="ot", tag="ot")
            nc.scalar.activation(ot, chunk_ps[key], AF.Identity,
                                 bias=bnf[oc], scale=NF)
            eng = ring or nc.gpsimd
            eng.dma_start(
                out=y[b, oc * 128:(oc + 1) * 128, r0:r0 + nr, :], in_=ot)
            del chunk_ps[key], chunk_cnt[key]

        def conv_chunk(b, oc, c, ring=None):
            for k in range(NTAP):
                for ic in range(2):
                    mm(b, oc, c, k, ic)
            drain_chunk(b, oc, c, ring=ring)

        NCH = (OH + ROW_CHUNK - 1) // ROW_CHUNK  # 8 chunks (last is 6 rows)

        # ---- prep0: oc0 stats/transposes fused with the first chunks'
        # matmuls.  Tap blocks cover c0/c1 in x-arrival order (ic0 piece
        # lands before ic1); the transpose for tap k+1 is tucked inside
        # tap k's block so its PSUM->SBUF copy hides under matmuls.  c2
        # runs as one block at the end (ic0 first: its piece lands ~1.5us
        # before the ic1 one).  Emission order = per-engine program order.
        stats_tap(0, 0)
        stats_tap(0, 1)
        nc.scalar.dma_start(out=xt[0][1][:, ROWS_B0[2][0]:ROWS_B0[2][1], :],
                            in_=x[0, 128:256, ROWS_B0[2][0]:ROWS_B0[2][1], :])
        stats_tap(0, 2)
        stats_tap(0, 3)
        nc.scalar.dma_start(out=xt[0][1][:, ROWS_B0[3][0]:ROWS_B0[3][1], :],
                            in_=x[0, 128:256, ROWS_B0[3][0]:ROWS_B0[3][1], :])
        stats_tap(0, 4)
        nc.scalar.dma_start(out=xt[0][1][:, ROWS_B0[4][0]:ROWS_B0[4][1], :],
                            in_=x[0, 128:256, ROWS_B0[4][0]:ROWS_B0[4][1], :])
        transpose_tap(0, 0)
        dummy(DUM_SPLICE)            # covers the wt0 PSUM->SBUF copy
        for k in range(NTAP):
            mm(0, 0, 0, k, 0)
            mm(0, 0, 0, k, 1)
            if k + 1 < NTAP:
                transpose_tap(0, k + 1)
            mm(0, 0, 1, k, 0)
            mm(0, 0, 1, k, 1)
        # c2 as one block, ic0 sweep then ic1 sweep
        for ic in range(2):
            for k in range(NTAP):
                mm(0, 0, 2, k, ic)
        # bnf0 on ACT before the first drain activation
        nc.scalar.mul(bnf[0], braw[0], NF)
        drain_chunk(0, 0, 0)
        drain_chunk(0, 0, 1)
        drain_chunk(0, 0, 2)
        # b1 bulk loads: logically delayed to ~12us so the scheduler can
        # never order their big copies ahead of the b0 pieces / W taps on
        # the serial copy engine (b1 is only consumed from ~40us).
        with tc.tile_wait_until(0.012):
            for (r0, r1) in ROWS_B1:
                nc.sync.dma_start(out=xt[1][0][:, r0:r1, :],
                                  in_=x[1, 0:128, r0:r1, :])
            for (r0, r1) in ROWS_B1:
                nc.scalar.dma_start(out=xt[1][1][:, r0:r1, :],
                                    in_=x[1, 128:256, r0:r1, :])
        nc.scalar.mul(bnf[1], braw[1], NF)

        # rest of b0.oc0, with oc1 stats/transposes tucked between chunks.
        # The oc1 stats are emitted here (not earlier): the Tile scheduler
        # reorders within a window, and oc1 ops emitted near the oc0 chain
        # get hoisted ahead of it on the in-order DVE/ACT engines, gating
        # everything on W.oc1's late arrival.
        for c in range(3, NCH):
            if c - 3 < NTAP:
                stats_tap(1, c - 3)
            conv_chunk(0, 0, c)
            if c - 3 < NTAP:
                transpose_tap(1, c - 3)

        for c in range(NCH):
            conv_chunk(0, 1, c)
        for c in range(NCH):
            conv_chunk(1, 0, c)
        for c in range(NCH):
            ring = None
            if c == NCH - 2:
                ring = nc.scalar
            elif c == NCH - 1:
                ring = nc.sync
            conv_chunk(1, 1, c, ring=ring)


def _build_nc():
    import concourse.mybir as mybir
    import concourse.tile as tile
    from concourse import bacc

    f32 = mybir.dt.float32
    f32r = mybir.dt.float32r
    nc = bacc.Bacc("TRN2", target_bir_lowering=False, debug=False)
    x = nc.dram_tensor("x", (B_LOCAL, IN_CH, H, H), f32r,
                       kind="ExternalInput").ap()
    w = nc.dram_tensor("w", (len(KERNEL_KEYS), OUT_CH, IN_CH), f32,
                       kind="ExternalInput").ap()
    bias = nc.dram_tensor("bias", (OUT_CH,), f32, kind="ExternalInput").ap()
    y = nc.dram_tensor("y", (B_LOCAL, OUT_CH, OH, OH), f32,
                       kind="ExternalOutput").ap()

    with tile.TileContext(nc) as tc:
        _emit(tc, nc, y, x, w, bias)
    nc.compile()
    return nc


def _get_nc():
    global _compiled_nc
    if _compiled_nc is None:
        _compiled_nc = _build_nc()
    return _compiled_nc


def _make_in_maps(x, W, bias):
    x = np.ascontiguousarray(x, dtype=np.float32)
    W = np.ascontiguousarray(W, dtype=np.float32)
    bias = np.ascontiguousarray(bias, dtype=np.float32)
    return [
        {
            "x": np.ascontiguousarray(x[i * B_LOCAL:(i + 1) * B_LOCAL]),
            "w": W,
            "bias": bias,
        }
        for i in range(N_CORES)
    ]


def kernel(x, W, bias):
    from concourse import bass_utils

    nc = _get_nc()
    res = bass_utils.run_bass_kernel_spmd(
        nc, _make_in_maps(x, W, bias), core_ids=list(range(N_CORES)))
    return np.concatenate([r["y"] for r in res.results], axis=0)
